# revision 5
# baseline (speedup 1.0000x reference)
"""Bass/Trainium2 kernel for CustomRNN (B=2048, T=512, I=1, H=64).

Math (per reference):
    xp[b,t,:] = x[b,t,0] * W_ih[:,0] + b_ih + b_hh
    h_{t+1}   = tanh(xp[:,t,:] + h_t @ W_hh.T),   h_0 = 0
    out       = h_T @ fc_w.T + fc_b              # [B, 1]

Sharding: pure data-parallel over batch. 8 cores x 256 rows; weights
replicated (baked into the NEFF as consts).

Per-core design (256 batch rows = 2 independent 128-col "chains"):
  - K=128 augmented matmul per step: the moving operand is a [128, 128]
    column block of a [128, 1024] bf16 "region" tile. Rows 0..63 hold
    h_t (updated in place), rows 64..127 statically hold the segment's
    64 x^T rows (x for steps s*64..s*64+63). The stationary lhsT is one
    of 64 pre-staged [128, 64] variants: rows 0..63 = W_hh^T, rows
    64..127 all zero except row 64+r = W_ih^T, which selects exactly
    step t's x row (r = t mod 64) at no extra cycles (matmul cost only
    depends on N). Every 64 steps the h window hops to the next 128-col
    segment (s = t div 64). Matmul operand base partitions stay 0, as
    the hardware requires.
  - One ScalarE ACT per chain-step does tanh(psum + bias) (bias is the
    per-partition [64,1] b_ih+b_hh) and writes bf16 h straight into the
    next read position. Two chains interleave so the matmul + sem
    latency of one chain hides under the other chain's ACT.
  - Final step writes h_T into a [65, 128] f32 tile whose last row is
    1.0; fc is one matmul with lhsT = [fc_w^T; fc_b] (K=65, M=1).
"""

import sys

if "/opt/trn_rl_repo" not in sys.path:
    sys.path.insert(0, "/opt/trn_rl_repo")

import ml_dtypes
import numpy as np

B, T, I, H = 2048, 512, 1, 64
N_CORES = 8
B_CORE = B // N_CORES          # 256
N_CHAINS = 2
BC = B_CORE // N_CHAINS        # 128 batch columns per chain
N_SEG = T // H                 # 8 column segments of 128 cols each
SEG_COLS = BC                  # 128
REG_COLS = N_SEG * SEG_COLS    # 1024

_CACHE = {}


def _build(weights):
    from concourse import bacc, mybir, tile

    lhsT_np, bias_np, fcT_np = weights

    nc = bacc.Bacc(None)
    f32 = mybir.dt.float32
    bf16 = mybir.dt.bfloat16

    xt_ext = nc.dram_tensor(
        "xt", [N_CHAINS, H, REG_COLS], bf16, kind="ExternalInput"
    )
    out_ext = nc.dram_tensor("out", [1, B_CORE], f32, kind="ExternalOutput")

    lhsT_dram = nc.inline_tensor(lhsT_np, name="lhsT_c")
    bias_dram = nc.inline_tensor(bias_np, name="bias_c")
    fcT_dram = nc.inline_tensor(fcT_np, name="fcT_c")

    with tile.TileContext(nc) as tc:
        with (
            tc.tile_pool(name="const", bufs=1) as cpool,
            tc.tile_pool(name="reg", bufs=1) as rpool,
            tc.tile_pool(name="fin", bufs=1) as fpool,
            tc.tile_pool(name="psA", bufs=2, space="PSUM") as psA,
            tc.tile_pool(name="psB", bufs=2, space="PSUM") as psB,
            tc.tile_pool(name="psF", bufs=2, space="PSUM") as psF,
        ):
            # 64 stationary variants, variant r at columns r*64..(r+1)*64.
            lhsT = cpool.tile([128, H * H], bf16, tag="lhsT")
            bias = cpool.tile([H, 1], f32, tag="bias")
            fcT = cpool.tile([H + 1, 1], f32, tag="fcT")
            nc.sync.dma_start(out=lhsT[:], in_=lhsT_dram[:])
            nc.sync.dma_start(out=bias[:], in_=bias_dram[:])
            nc.sync.dma_start(out=fcT[:], in_=fcT_dram[:])

            regions = []
            fins = []
            for c in range(N_CHAINS):
                reg = rpool.tile([128, REG_COLS], bf16, tag=f"reg{c}")
                # x^T rows for every step, pre-staged under the h window.
                nc.sync.dma_start(out=reg[H:128, :], in_=xt_ext[c])
                # h_0 = 0 for step 0's window (partitions 0..63, seg 0).
                nc.vector.memset(reg[0:H, 0:SEG_COLS], 0.0)
                regions.append(reg)

                fin = fpool.tile([H + 1, BC], f32, tag=f"fin{c}")
                nc.vector.memset(fin[H : H + 1, :], 1.0)
                fins.append(fin)

            psum_pools = [psA, psB]
            tanh = mybir.ActivationFunctionType.Tanh
            for t in range(T):
                s, r = divmod(t, H)
                c0 = s * SEG_COLS
                c1 = ((t + 1) // H) * SEG_COLS
                for c in range(N_CHAINS):
                    reg = regions[c]
                    ps = psum_pools[c].tile([H, BC], f32, tag=f"ps{c}")
                    nc.tensor.matmul(
                        out=ps[:],
                        lhsT=lhsT[:, r * H : (r + 1) * H],
                        rhs=reg[:, c0 : c0 + SEG_COLS],
                        start=True,
                        stop=True,
                    )
                    if t + 1 < T:
                        dst = reg[0:H, c1 : c1 + SEG_COLS]
                    else:
                        dst = fins[c][0:H, :]
                    nc.scalar.activation(dst, ps[:], tanh, bias=bias[:])

            # fc: out[b] = fc_w . h_T[b] + fc_b  via K=65 matmul with 1-row.
            out_sb = cpool.tile([1, B_CORE], f32, tag="out_sb")
            for c in range(N_CHAINS):
                pf = psF.tile([1, BC], f32, tag="pf")
                nc.tensor.matmul(
                    out=pf[:], lhsT=fcT[:], rhs=fins[c][:], start=True, stop=True
                )
                nc.vector.tensor_copy(out_sb[:, c * BC : (c + 1) * BC], pf[:])
            nc.sync.dma_start(out=out_ext[:], in_=out_sb[:])

    nc.finalize()
    return nc


def _prep_weights(W_ih, W_hh, b_ih, b_hh, fc_w, fc_b):
    bf16 = ml_dtypes.bfloat16
    # [128, 64*64]: variant r at cols r*64..(r+1)*64; rows 0..63 = W_hh^T,
    # rows 64..127 zero except row 64+r = W_ih^T.
    lhsT = np.zeros((128, H * H), np.float32)
    for r in range(H):
        lhsT[0:H, r * H : (r + 1) * H] = W_hh.T
        lhsT[H + r, r * H : (r + 1) * H] = W_ih[:, 0]
    lhsT = lhsT.astype(bf16)
    bias = (b_ih + b_hh).astype(np.float32).reshape(H, 1)         # [64, 1]
    fcT = np.concatenate(
        [fc_w.reshape(H, 1), fc_b.reshape(1, 1)], axis=0
    ).astype(np.float32)                                          # [65, 1]
    return lhsT, bias, fcT


def _prep_x(x):
    """Per-core, per-chain staged x^T: [cores][chain, 64, 1024] bf16.

    stage[c][ch, r, s*128 + j] = x[c*256 + ch*128 + j, s*64 + r]
    """
    xf = np.ascontiguousarray(x.reshape(B, T))
    out = []
    for c in range(N_CORES):
        chains = []
        for ch in range(N_CHAINS):
            xc = xf[c * B_CORE + ch * BC : c * B_CORE + (ch + 1) * BC]  # [128, 512]
            st = (
                xc.reshape(BC, N_SEG, H)
                .transpose(2, 1, 0)
                .reshape(H, REG_COLS)
            )
            chains.append(st)
        out.append(
            np.stack(chains, axis=0).astype(ml_dtypes.bfloat16)
        )
    return out


def kernel(x, W_ih, W_hh, b_ih, b_hh, fc_w, fc_b):
    from concourse.bass_utils import run_bass_kernel_spmd

    x = np.asarray(x, np.float32)
    key = "nc"
    if key not in _CACHE:
        _CACHE[key] = _build(
            _prep_weights(
                np.asarray(W_ih, np.float32),
                np.asarray(W_hh, np.float32),
                np.asarray(b_ih, np.float32),
                np.asarray(b_hh, np.float32),
                np.asarray(fc_w, np.float32),
                np.asarray(fc_b, np.float32),
            )
        )
    nc = _CACHE[key]

    in_maps = [{"xt": xt} for xt in _prep_x(x)]
    res = run_bass_kernel_spmd(nc, in_maps, list(range(N_CORES)))
    out = np.concatenate(
        [np.asarray(res.results[c]["out"][0], np.float32) for c in range(N_CORES)]
    )
    return out.reshape(B, 1)


if __name__ == "__main__":
    rng = np.random.default_rng(0)
    s = 1.0 / np.sqrt(H)
    inputs = {
        "x": rng.standard_normal((B, T, I)).astype(np.float32),
        "W_ih": rng.uniform(-s, s, (H, I)).astype(np.float32),
        "W_hh": rng.uniform(-s, s, (H, H)).astype(np.float32),
        "b_ih": rng.uniform(-s, s, H).astype(np.float32),
        "b_hh": rng.uniform(-s, s, H).astype(np.float32),
        "fc_w": rng.uniform(-s, s, (1, H)).astype(np.float32),
        "fc_b": rng.uniform(-s, s, 1).astype(np.float32),
    }
    out = kernel(**inputs)
    print("kernel out", out.shape, out[:4, 0])
